# revision 37
# baseline (speedup 1.0000x reference)
"""Constraint-projection layer on 8 Trainium2 NeuronCores.

Reference computes, per batch row y_i:  x_i = argmin ||x - y_i|| s.t. A x = b_i
via a dense KKT solve. Closed form (Schur complement of the KKT system):

    x = y - A^T G (A y - b),   G = (A A^T)^{-1}  (host-precomputed, 128x128)

Each core gets a 2048-row batch shard in TRANSPOSED layout (dim-major), so
all matmuls contract over the partition axis with contiguous DMA only:

    stage 1:  V = A @ Y^T - B^T            (128 m  x 512 batch tiles)
    stage 2:  U = G @ V                    (128 m  x 512)
    stage 3:  X^T = Y^T - A^T_chunk @ U    (1024 d x 512)

A is loaded once in its native (m, d) layout — stage 3 uses it directly as
the stationary operand; stage 1's A^T chunks are produced on-core by PE
transposes, saving the separate 0.5 MiB A^T (and 0.5 MiB W) HBM loads. The
kernel is DMA-roofline-bound: ~17.6 MiB/core crosses HBM at the modeled
360 GB/s, so the schedule keeps the (serially-modeled) DMA engines packed:
all loads issue up front on the sync ring, stores ride the scalar ring.

Data-parallel: no cross-core communication.
"""

import os

import numpy as np
import bass_rust as _br
import concourse.bass as bass
import concourse.mybir as mybir
from concourse import masks, tile
from concourse.bass_utils import run_bass_kernel_spmd

F32 = mybir.dt.float32
F32R = mybir.dt.float32r
# fp32r streams through the PE at 4x the fp32 rate (1 cycle/row vs 4) at
# slightly reduced multiply precision. The correctness gate is rel_err < 2e-2
# and all-f32r measures ~1e-4, while fp32 matmuls make the PE the
# store-production bottleneck — so default everything to f32r.
# Modes: "f32", "f32r", "hybrid1" (stage1 f32r), "hybrid2" (stages 2+3 f32r).
MM_MODE = os.environ.get("KERNEL_MM_MODE", "f32r")
_S1_R = MM_MODE in ("f32r", "hybrid1")
_S2_R = MM_MODE in ("f32r", "hybrid2")
BF16 = mybir.dt.bfloat16
# The y/b batch streams are loaded as bf16 (host-side cast). The correctness
# gate is rel_err < 2e-2; the bf16 perturbation passes through the orthogonal
# projector x = x* + (I-P)dy, so the result error stays at the ~2^-9 input
# rounding level (~2e-3 measured). This halves the dominant input DMA
# traffic on the serialized DMA engines. KERNEL_IN_DTYPE=f32 restores
# full-precision inputs.
IN_DTYPE = os.environ.get("KERNEL_IN_DTYPE", "bf16")
_IN16 = IN_DTYPE == "bf16"
DT_IN = BF16 if _IN16 else F32
# The x output is produced as bf16 on-core and upcast to f32 on the host:
# output quantization adds ~2e-3 rel err on top of the input rounding
# (~3.6e-3 combined, gate is 2e-2) and halves the store traffic, which
# dominates once the inputs are bf16. KERNEL_OUT_DTYPE=f32 restores
# full-precision stores.
OUT_DTYPE = os.environ.get("KERNEL_OUT_DTYPE", "bf16")
_OUT16 = OUT_DTYPE == "bf16"
DT_OUT = BF16 if _OUT16 else F32


def _s1(ap):
    # stage-1 operand view: bf16 tiles are already a fast matmul dtype
    return ap if _IN16 else (ap.bitcast(F32R) if _S1_R else ap)


def _s2(ap):
    return ap.bitcast(F32R) if _S2_R else ap

N_CORES = 8
BATCH = 16384
N = 1024           # input dim
M = 128            # constraint dim
BC = BATCH // N_CORES  # 2048 batch rows per core
KC = N // 128      # 8 contraction chunks
F = 512            # free-dim tile (one PSUM bank of f32)
NJ = BC // F       # 4 batch tiles per core


def _split_drain_and_barrier(self, tick_clock, wait_clock):
    # Walrus in this toolchain rejects >2 sync waits on the Tile tail Drain
    # (CTRL_NO_STRUCT). Emit one-wait-per-nop instructions ahead of the
    # drain instead; sequentially identical on the sync sequencer.
    gc = tick_clock.global_clock
    vals = eval(repr(gc).replace("VectorClock", "").strip("()"))
    for i, v in enumerate(vals):
        if v:
            single = [0] * len(vals)
            single[i] = v
            nop = self.nc.sync.nop(nofuse=True)
            wait_clock.add_sem_waits(
                nop.ins, _br.ScopedClock({None: _br.VectorClock(single)})
            )
    self.nc.sync.drain()
    if os.environ.get("KERNEL_TAIL_BARRIER", "0") == "1":
        self.nc.all_engine_barrier()
    assert self.sems is not None
    popped = self.nc._tile_sem_poison_stack.pop()
    assert popped is self._sem_poison
    if os.environ.get("KERNEL_FULL_TEARDOWN", "0") == "1":
        self.nc.clear_and_free_semaphores(list(self.sems.allocated().values()))
        self.nc.all_engine_barrier()
    else:
        # Entry re-initializes every semaphore (RegisterMove/Memset preamble
        # runs on each launch), so the teardown sem/DGE clear + second
        # barrier are redundant; keep only the allocator bookkeeping.
        sems = list(self.sems.allocated().values())
        sem_nums = [s.num for s in sems]
        self.nc._state.prepend_free_semaphores(sem_nums)
        for poison_set in self.nc._tile_sem_poison_stack:
            poison_set.update(sem_nums)


tile.TileContext._drain_and_barrier = _split_drain_and_barrier

_orig_commit_and_lower = tile.TileContext._commit_and_lower

# Same walrus limitation for regular instructions: Matmult (S3_LW) takes no
# extra sync waits, most others take one. Spill excess waits onto dedicated
# same-engine nops committed immediately before the instruction.
_ZERO_WAIT_OPS = ("InstMatmult", "InstDrain")


def _split_commit_and_lower(self, inst, original_block, old_bb_map, bb_to_exit_bb):
    tn = type(inst).__name__
    if tn.startswith("Inst") and inst.engine is not None:
        si = inst.sync_info
        if si is not None:
            waits = list(si.on_wait)
            keep = 0 if tn in _ZERO_WAIT_OPS else 1
            if len(waits) > keep:
                spill, keep_waits = (
                    (waits, []) if keep == 0 else (waits[:-1], [waits[-1]])
                )
                for w_ in spill:
                    nop = mybir.InstNoOp(
                        name=self.nc.get_next_instruction_name(),
                        engine=inst.engine,
                        sync_info=mybir.SyncInfo(on_wait=[w_], on_update=[]),
                        bass_nofuse=True,
                    )
                    self._commit_instruction(nop)
                inst.sync_info = mybir.SyncInfo(
                    on_wait=keep_waits, on_update=list(si.on_update)
                )
    return _orig_commit_and_lower(self, inst, original_block, old_bb_map, bb_to_exit_bb)


tile.TileContext._commit_and_lower = _split_commit_and_lower


# Bass.__init__ ends with const-scalar-tile memsets (f32 0/1, bf16 1, u8 127)
# plus an all-engine barrier before the program block. Nothing in this kernel
# reads those const tiles (Copy-activation bias stays an immediate; DVE
# tensor ops and matmuls take no scalar APs), and semaphore state is
# runtime-reset per launch, so the entry barrier orders nothing observable —
# but it delays the first DMA issue by ~0.7us. Skip exactly that first
# barrier; every later barrier (the teardown drain) passes through.
_orig_barrier = bass.Bass.all_engine_barrier


def _skip_init_barrier(self, **kw):
    if getattr(self, "_init_barrier_pending", True):
        self._init_barrier_pending = False
        if os.environ.get("KERNEL_INIT_BARRIER", "0") != "1":
            return None
    return _orig_barrier(self, **kw)


bass.Bass.all_engine_barrier = _skip_init_barrier


def build_nc() -> bass.Bass:
    nc = bass.Bass()
    yt_d = nc.declare_dram_parameter("yt", [N, BC], DT_IN, isOutput=False)
    bt_d = nc.declare_dram_parameter("bt", [M, BC], DT_IN, isOutput=False)
    a_d = nc.declare_dram_parameter("a", [M, N], F32, isOutput=False)
    g_d = nc.declare_dram_parameter("g", [M, M], F32, isOutput=False)
    out_d = nc.declare_dram_parameter("out", [N, BC], DT_OUT, isOutput=True)

    # dim-chunked 3D views: partition = row-within-chunk, then (chunk, batch)
    yt_v = yt_d.rearrange("(k p) b -> p k b", p=128)
    out_v = out_d.rearrange("(k p) b -> p k b", p=128)

    with tile.TileContext(nc) as tc:
        with (
            tc.tile_pool(name="const", bufs=1) as constp,
            tc.tile_pool(name="yts", bufs=NJ) as ytp,
            tc.tile_pool(name="tts", bufs=2) as ttp,
            tc.tile_pool(name="bts", bufs=NJ) as btp,
            tc.tile_pool(name="us", bufs=2) as usp,
            tc.tile_pool(name="outs", bufs=8) as outp,
            tc.tile_pool(name="ps1", bufs=2, space="PSUM") as ps1,
            tc.tile_pool(name="ps2", bufs=3, space="PSUM") as ps2,
        ):
            # All input loads issue up front on the sync ring, ordered so the
            # DMA engines never idle and tile-0 compute starts ASAP:
            # A, Y0, G, B, Y1, Y2, Y3. Stores later ride the scalar ring.
            a_s = constp.tile([128, N], F32)  # A: partition = m, free = dim
            nc.sync.dma_start(_s2(a_s[:]), _s2(a_d[:]))
            ytjs = []

            def load_yt(j):
                ytj = ytp.tile([128, KC, F], DT_IN)
                nc.sync.dma_start(
                    _s1(ytj[:]), _s1(yt_v[:, :, j * F:(j + 1) * F])
                )
                ytjs.append(ytj)

            # b arrives as per-tile slices interleaved after each y tile:
            # tile j's -b accumulation only needs slice j, and the later y
            # tiles then land ~1us earlier, which feeds the PE sooner.
            btjs = []

            def load_bt(j):
                btj = btp.tile([128, F], DT_IN)
                nc.sync.dma_start(btj[:], bt_d[:, j * F:(j + 1) * F])
                btjs.append(btj)

            load_yt(0)
            load_bt(0)
            g_s = constp.tile([128, M], F32)  # G: partition = m, free = m
            nc.sync.dma_start(_s2(g_s[:]), _s2(g_d[:]))
            for j in range(1, NJ):
                load_yt(j)
                load_bt(j)

            # On-core A^T for stage 1: PE-transpose A's eight 128x128 chunks
            # (identity built on the otherwise-idle gpsimd engine), then
            # scalar-engine copies PSUM -> SBUF. Replaces a 0.5 MiB HBM load
            # on the serialized DMA engines with idle-engine work. Batches of
            # 4 chunks so the scratch fits a single PSUM bank (the psg pool).
            # The transpose chain stays plain fp32 (fp32r Memset/transpose are
            # not encodable ISA); the DVE copy PSUM -> SBUF is the fp32r
            # "rounding" producer the walrus BIR verifier requires for data
            # consumed by fp32r matmuls.
            # PE P-state warm-up: the tensor engine clocks up only after
            # ~3us of continuous execution. A burst of tiny matmuls on a
            # zeroed scratch tile (issued before A lands) ramps the clock so
            # the transposes and tile-0 mm1 run at full rate instead of the
            # 2-4x slower cold rate, pulling the whole left edge of the
            # pipeline forward.
            warm = constp.tile([128, 64], F32)
            nc.gpsimd.memset(warm[:], 0.0)
            pw = ps2.tile([128, 2, F], F32, tag="p2")
            for w in range(14):
                nc.tensor.matmul(
                    pw[:64, 0, :64], warm[:, :64], warm[:], start=True,
                    stop=True,
                )
            id_s = constp.tile([128, 128], F32)
            masks.make_identity(nc, id_s[:])
            # -I in the input dtype: stage 1's ninth matmul accumulates
            # -1 * B^T into the same PSUM group, removing the DVE v-subtract
            # from the serial store-production chain.
            nid_s = constp.tile([128, 128], DT_IN)
            nc.gpsimd.memset(nid_s[:], 0.0)
            nc.gpsimd.affine_select(
                out=nid_s[:],
                in_=nid_s[:],
                compare_op=mybir.AluOpType.not_equal,
                fill=-1.0,
                base=0,
                pattern=[[-1, 128]],
                channel_multiplier=1,
            )
            at_s = constp.tile([128, KC, M], DT_IN)  # A^T: p = dim, free = m
            for half in range(2):
                # shares the ps1 slot group with the stage-1 accumulator
                # (same 2 KiB/partition footprint, disjoint lifetime)
                pt_t = ps1.tile([128, 4, 128], F32, tag="acc")
                for k in range(4):
                    kk = half * 4 + k
                    nc.tensor.transpose(
                        pt_t[:, k, :], a_s[:, kk * 128:(kk + 1) * 128], id_s[:]
                    )
                nc.scalar.copy(
                    _s1(at_s[:, half * 4:(half + 1) * 4, :]), pt_t[:]
                )

            # Software pipeline with a one-tile skew. Each engine's sequencer
            # is in-order, so prep(j+1) -- the serial mm1 -> v-sub -> mmG ->
            # u-copy latency chain -- is issued BEFORE emit(j)'s stage-3
            # matmuls/subs/stores; the next tile's chain advances while the
            # previous tile drains to HBM, keeping the store stream packed.
            uss = [None] * NJ

            tts = [None] * NJ

            def prep_a(j):
                # stage 1: V = A Y^T - B^T (the -b rides the same PSUM
                # accumulation group via the -I stationary), then the Act
                # copy moves V to SBUF for stage 2's moving operand.
                ytj = ytjs[j]
                pt = ps1.tile([128, F], F32, tag="acc")
                for k in range(KC):
                    nc.tensor.matmul(
                        pt[:],
                        _s1(at_s[:, k, :]),
                        _s1(ytj[:, k, :]),
                        start=(k == 0),
                        stop=False,
                    )
                nc.tensor.matmul(
                    pt[:],
                    _s1(nid_s[:]),
                    _s1(btjs[j][:]),
                    start=False,
                    stop=True,
                )
                tt = ttp.tile([128, F], F32)
                nc.scalar.copy(_s2(tt[:]), pt[:])
                tts[j] = tt

            def prep_b(j):
                # pu rotates through the ps1 slot group (disjoint lifetime
                # with pt: pt is dead once the V copy reads it), freeing two
                # PSUM banks for a third stage-3 slot.
                pu = ps1.tile([128, F], F32, tag="acc")  # U = G V
                nc.tensor.matmul(
                    pu[:], _s2(g_s[:]), _s2(tts[j][:]), start=True, stop=True
                )
                us = usp.tile([128, F], F32)
                nc.scalar.copy(_s2(us[:]), pu[:])
                uss[j] = us

            def emit(j, h):
                # stage 3 for one half (4 d-chunks) of tile j, in pairs of
                # d-chunks (2-bank PSUM tiles). Pairs are split across
                # engines so no single engine paces store production:
                #   DVE pair: p2 = A^T u      -> DVE sub   oh = y - p2
                #   PE  pair: p2 = A^T u - y  -> Act copy  oh = -1 * p2
                # (the -y accumulation reuses the -I stationary; the Act copy
                # also performs the f32 PSUM -> bf16 downconvert). One PE
                # pair per tile (h=1, g=1): DVE has slack, PE is the pacer.
                ytj = ytjs[j]
                us = uss[j]
                oh = outp.tile([128, KC // 2, F], DT_OUT)
                for g in range(2):  # two d-pairs per half
                    # late tiles shift a second pair to PE+Act so the DVE
                    # queue is not the last producer before the final stores
                    on_pe = g == 1 and (h == 1 or j >= NJ - 2)
                    p2 = ps2.tile([128, 2, F], F32, tag="p2")
                    for e in range(2):
                        d = h * 4 + g * 2 + e
                        nc.tensor.matmul(
                            p2[:, e, :],
                            _s2(a_s[:, d * 128:(d + 1) * 128]),
                            _s2(us[:]),
                            start=True,
                            stop=not on_pe,
                        )
                    d0 = h * 4 + g * 2
                    if not on_pe:
                        nc.vector.tensor_sub(
                            oh[:, g * 2:(g + 1) * 2, :],
                            ytj[:, d0:d0 + 2, :],
                            p2[:],
                        )
                    else:
                        for e in range(2):
                            d = h * 4 + g * 2 + e
                            nc.tensor.matmul(
                                p2[:, e, :],
                                _s1(nid_s[:]),
                                _s1(ytj[:, d, :]),
                                start=False,
                                stop=True,
                            )
                        nc.scalar.mul(
                            oh[:, g * 2:(g + 1) * 2, :], p2[:], -1.0
                        )
                # stores ride the sync (SP) ring: SP is idle once the loads
                # have issued, so a store's sem wait never head-of-line
                # blocks the scalar engine's tt/us copies (and all loads are
                # already enqueued, so ring FIFO order costs nothing).
                nc.sync.dma_start(
                    out_v[:, h * 4:(h + 1) * 4, j * F:(j + 1) * F], oh[:]
                )

            # Two-deep skew: tile j+1's mm1 block runs while tile j's U
            # copy round-trips through the Act engine, and tile j's stage-3
            # fills tile j+1's V-copy window — the PE stream never stalls
            # on the PSUM->SBUF copy latency.
            prep_a(0)
            prep_b(0)
            prep_a(1)
            emit(0, 0)
            prep_b(1)
            for j in range(1, NJ - 1):
                prep_a(j + 1)
                emit(j - 1, 1)
                emit(j, 0)
                prep_b(j + 1)
            emit(NJ - 2, 1)
            emit(NJ - 1, 0)
            emit(NJ - 1, 1)
    return nc


_NC_CACHE = None
_RUNNER = None


def _get_nc():
    global _NC_CACHE
    if _NC_CACHE is None:
        _NC_CACHE = build_nc()
    return _NC_CACHE


def _build_runner():
    """Persistent jitted shard_map callable over 8 cores (mirrors
    bass2jax.run_bass_via_pjrt's multi-core path, but cached so repeated
    kernel() calls skip retracing/XLA recompile)."""
    import jax
    from jax.sharding import Mesh, PartitionSpec
    from jax.experimental.shard_map import shard_map
    from concourse import bass2jax as b2j

    nc = _get_nc()
    b2j.install_neuronx_cc_hook()
    assert nc.dbg_addr is None
    partition_name = nc.partition_id_tensor.name if nc.partition_id_tensor else None

    in_names, out_names, out_avals, zero_shapes = [], [], [], []
    for alloc in nc.m.functions[0].allocations:
        if not isinstance(alloc, mybir.MemoryLocationSet):
            continue
        name = alloc.memorylocations[0].name
        if alloc.kind == "ExternalInput":
            if name != partition_name:
                in_names.append(name)
        elif alloc.kind == "ExternalOutput":
            out_names.append(name)
            shape = tuple(alloc.tensor_shape)
            dtype = mybir.dt.np(alloc.dtype)
            out_avals.append(jax.core.ShapedArray(shape, dtype))
            zero_shapes.append((shape, dtype))
    n_params = len(in_names)
    n_outs = len(out_names)
    all_in_names = tuple(in_names) + tuple(out_names)
    if partition_name is not None:
        all_in_names = all_in_names + (partition_name,)

    def _body(*args):
        operands = list(args)
        if partition_name is not None:
            operands.append(b2j.partition_id_tensor())
        outs = b2j._bass_exec_p.bind(
            *operands,
            out_avals=tuple(out_avals),
            in_names=all_in_names,
            out_names=tuple(out_names),
            lowering_input_output_aliases=(),
            sim_require_finite=True,
            sim_require_nnan=True,
            nc=nc,
        )
        return tuple(outs)

    devices = jax.devices()[:N_CORES]
    mesh = Mesh(np.asarray(devices), ("core",))
    in_specs = (PartitionSpec("core"),) * (n_params + n_outs)
    out_specs = (PartitionSpec("core"),) * n_outs
    donate = tuple(range(n_params, n_params + n_outs))
    sharded = jax.jit(
        shard_map(
            _body, mesh=mesh, in_specs=in_specs, out_specs=out_specs,
            check_rep=False,
        ),
        donate_argnums=donate,
        keep_unused=True,
    )

    from jax.sharding import NamedSharding

    zeros_fns = [
        jax.jit(
            lambda s=shape, d=dtype: jax.numpy.zeros(
                (N_CORES * s[0], *s[1:]), d
            ),
            out_shardings=NamedSharding(mesh, PartitionSpec("core")),
        )
        for shape, dtype in zero_shapes
    ]

    def run(named_inputs: dict):
        """named_inputs: name -> concatenated (N_CORES*dim0, ...) array."""
        ins = [named_inputs[n] for n in in_names]
        zeros = [f() for f in zeros_fns]
        outs = sharded(*ins, *zeros)
        return dict(zip(out_names, outs))

    run._parts = {
        "sharded": sharded,
        "in_names": in_names,
        "out_names": out_names,
        "mesh": mesh,
        "zeros_fns": zeros_fns,
    }
    return run


def _get_runner():
    global _RUNNER
    if _RUNNER is None:
        _RUNNER = _build_runner()
    return _RUNNER


def _prep_inputs(y, A, b):
    A64 = A.astype(np.float64)
    G = np.linalg.inv(A64 @ A64.T).astype(np.float32)  # (M, M), symmetric
    np_in = mybir.dt.np(DT_IN)
    # concat-over-cores layouts expected by the shard_map runner
    yt_cat = np.ascontiguousarray(
        y.reshape(N_CORES, BC, N).transpose(0, 2, 1).astype(np_in)
    ).reshape(N_CORES * N, BC)
    bt_cat = np.ascontiguousarray(
        b.reshape(N_CORES, BC, M).transpose(0, 2, 1).astype(np_in)
    ).reshape(N_CORES * M, BC)
    a_cat = np.broadcast_to(A, (N_CORES, M, N)).reshape(N_CORES * M, N)
    g_cat = np.broadcast_to(G, (N_CORES, M, M)).reshape(N_CORES * M, M)
    return {"yt": yt_cat, "bt": bt_cat, "a": a_cat, "g": g_cat}


def _unpack_output(out_cat: np.ndarray) -> np.ndarray:
    return np.ascontiguousarray(
        np.asarray(out_cat).astype(np.float32)
        .reshape(N_CORES, N, BC).transpose(0, 2, 1)
    ).reshape(BATCH, N)


def kernel(y: np.ndarray, A: np.ndarray, b: np.ndarray) -> np.ndarray:
    y = np.ascontiguousarray(np.asarray(y, dtype=np.float32))
    A = np.ascontiguousarray(np.asarray(A, dtype=np.float32))
    b = np.ascontiguousarray(np.asarray(b, dtype=np.float32))
    assert y.shape == (BATCH, N) and A.shape == (M, N) and b.shape == (BATCH, M)

    named = _prep_inputs(y, A, b)
    try:
        run = _get_runner()
        out = run(named)["out"]
        return _unpack_output(out)
    except Exception:
        # Fallback: slower but uses only the public SPMD entry point.
        in_maps = [
            {
                k: np.ascontiguousarray(
                    v.reshape(N_CORES, v.shape[0] // N_CORES, *v.shape[1:])[i]
                )
                for k, v in named.items()
            }
            for i in range(N_CORES)
        ]
        res = run_bass_kernel_spmd(_get_nc(), in_maps, list(range(N_CORES)))
        x = np.empty((BATCH, N), dtype=np.float32)
        for i in range(N_CORES):
            x[i * BC:(i + 1) * BC, :] = res.results[i]["out"].T
        return x
